# revision 16
# baseline (speedup 1.0000x reference)
"""Additive-attention pooling kernel for TRN2, data-parallel over batch on 8 cores.

Reference computation (per batch b):
    att[s, o]  = tanh(sum_h x[s,b,h] * W[o,h] + bias[o])
    sim[s]     = sum_o att[s, o] * context[o]
    e[s]       = exp(sim[s])            (softmax without max-subtraction;
                                         |sim| <= sum|ctx| <= 22.6 so exp is safe)
    out[b, h]  = sum_s x[s,b,h] * e[s] / sum_s e[s]

V3 dataflow (4 batches/core, pair-units of ST2=1024 seq positions):
  - GEMM: fp8e4m3 DoubleRow matmuls (2 k-chunks per instruction, 2x bf16
    throughput) into a [128, 1024] 2-bank psum per oc; one ACT Tanh per oc
    evacuates the full 1024-wide tile with the per-oc bias (bf16 out).
  - sim: stationary is ctx replicated across 128 columns, so the sim psum
    [128, 1024] holds sim broadcast to every partition.  One ACT Exp
    evacuates it to bf16 e_bc AND emits the denominator partial via
    accum_out - no separate broadcast step.
  - pooling: DVE affine_mul_reduce (fused mul+reduce; tensor_tensor_reduce
    crashes real HW) of x16 chunks against e_bc, accum_out -> num partials.
  - epilogue per batch: reduce partials, reciprocal, scale, DMA out.
"""

import sys

if "/opt/trn_rl_repo" not in sys.path:
    sys.path.insert(0, "/opt/trn_rl_repo")

import numpy as np

SEQ, BATCH, HID = 4096, 32, 512
NCORES = 8
BPC = BATCH // NCORES          # batches per core
ST2 = 1024                     # pair-unit seq width (2 psum banks)
NP = SEQ // ST2                # 4 pair-units per batch
NHC = HID // 128               # 4 h chunks
NOC = HID // 128               # 4 o chunks

# "fp8": x8/W8 e4m3 DoubleRow GEMM (+bf16 x copy for pooling)
# "bf16": bf16 GEMM reading the pooling x copy (no x8)
GEMM_MODE = "fp8"

_CACHE = {}


def _build(mode):
    import concourse.tile as tile
    from concourse import bacc, mybir

    F32 = mybir.dt.float32
    BF16 = mybir.dt.bfloat16
    FP8 = mybir.dt.float8e4
    AF = mybir.ActivationFunctionType
    PM = mybir.MatmulPerfMode

    nc = bacc.Bacc(
        "TRN2",
        target_bir_lowering=False,
        debug=False,
        enable_asserts=True,
        num_devices=NCORES,
    )

    # [h_lo 128, b, hc, s]
    x16_d = nc.dram_tensor("x16", (128, BPC, NHC, SEQ), BF16, kind="ExternalInput").ap()
    if mode == "fp8":
        x8_d = nc.dram_tensor("x8", (128, BPC, NHC, SEQ), FP8, kind="ExternalInput").ap()
        wt_d = nc.dram_tensor("wt", (128, NHC, HID), FP8, kind="ExternalInput").ap()
    else:
        wt_d = nc.dram_tensor("wt", (128, NHC, HID), BF16, kind="ExternalInput").ap()
    b_d = nc.dram_tensor("bvec", (128, NOC), F32, kind="ExternalInput").ap()
    # ctx replicated along free dim: ctxrep[p, oc*128 + m] = ctx[oc*128 + p]
    ctxrep_d = nc.dram_tensor("ctxrep", (128, NOC * 128), BF16, kind="ExternalInput").ap()
    # [b, p, hc] so each batch's out DMA is one contiguous 2KB descriptor;
    # host reassembles out[b, hc*128+p] = out_host[b, p, hc]
    out_d = nc.dram_tensor("out", (BPC, 128, NHC), F32, kind="ExternalOutput").ap()

    with tile.TileContext(nc) as tc:
        with (
            tc.tile_pool(name="consts", bufs=1) as cpool,
            tc.tile_pool(name="xs", bufs=3) as xpool,
            tc.tile_pool(name="acts", bufs=2) as apool,
            tc.tile_pool(name="es", bufs=2) as epool,
            tc.tile_pool(name="junks", bufs=2) as jpool,
            tc.tile_pool(name="small", bufs=2) as spool,
            tc.tile_pool(name="attps", bufs=2, space="PSUM") as ps_att,
            tc.tile_pool(name="simps", bufs=2, space="PSUM") as ps_sim,
        ):
            wdt = FP8 if mode == "fp8" else BF16
            wt_sb = cpool.tile([128, NHC, HID], wdt, tag="wt")
            b_sb = cpool.tile([128, NOC], F32, tag="b")
            ctx_sb = cpool.tile([128, NOC * 128], BF16, tag="ctx")
            nc.sync.dma_start(wt_sb[:], wt_d)
            nc.sync.dma_start(b_sb[:], b_d)
            nc.sync.dma_start(ctx_sb[:], ctxrep_d)

            # PE pstate warm-up: ~12 junk matmuls on a zeroed tile so the
            # first real matmuls run at full clock (PE ramps over ~3us)
            warm = cpool.tile([128, 512], BF16, tag="warm")
            nc.vector.memset(warm[:], 0.0)
            for i in range(6):
                warmps = ps_sim.tile([128, ST2], F32, tag="sim")
                nc.tensor.matmul(
                    warmps[:, 0:512], warm[:, 0:128], warm[:], start=True, stop=True
                )

            for b in range(BPC):
                # chunk the x DMAs per pair-unit so the first att matmuls can
                # start as soon as the first quarter lands (subtile deps);
                # issue from the idle GpSimd queue (cheap issue; the sync
                # queue's DIRECT2D costs ~770ns serial per dma_start)
                x16 = xpool.tile([128, NHC, SEQ], BF16, tag="x16")
                if mode == "fp8":
                    x8 = xpool.tile([128, NHC, SEQ], FP8, tag="x8")
                    # finer leading chunks on batch 0 so compute starts sooner
                    bounds = (
                        [0, 256, 512, 1024, 2048, 3072, 4096]
                        if b == 0
                        else [0, 1024, 2048, 3072, 4096]
                    )
                    for lo, hi in zip(bounds, bounds[1:]):
                        csl = slice(lo, hi)
                        nc.gpsimd.dma_start(x8[:, :, csl], x8_d[:, b, :, csl])
                for u in range(NP):
                    ssl2 = slice(u * ST2, (u + 1) * ST2)
                    nc.gpsimd.dma_start(x16[:, :, ssl2], x16_d[:, b, :, ssl2])

                num_cols = spool.tile([128, NHC * NP], F32, tag="num")
                den_cols = spool.tile([128, NP], F32, tag="den")

                for u in range(NP):
                    ssl2 = slice(u * ST2, (u + 1) * ST2)
                    acts = apool.tile([128, NOC, ST2], BF16, tag="act")
                    for oc in range(NOC):
                        attps = ps_att.tile([128, ST2], F32, tag="att")
                        for half in range(2):
                            osl = slice(half * 512, (half + 1) * 512)
                            ssl = slice(u * ST2 + half * 512, u * ST2 + (half + 1) * 512)
                            if mode == "fp8":
                                for j in range(2):
                                    nc.tensor.matmul(
                                        attps[:, osl],
                                        wt_sb[:, 2 * j : 2 * j + 2,
                                              oc * 128 : (oc + 1) * 128],
                                        x8[:, 2 * j : 2 * j + 2, ssl],
                                        start=(j == 0),
                                        stop=(j == 1),
                                        perf_mode=PM.DoubleRow,
                                    )
                            else:
                                for hc in range(NHC):
                                    nc.tensor.matmul(
                                        attps[:, osl],
                                        wt_sb[:, hc, oc * 128 : (oc + 1) * 128],
                                        x16[:, hc, ssl],
                                        start=(hc == 0),
                                        stop=(hc == NHC - 1),
                                    )
                        nc.scalar.activation(
                            acts[:, oc], attps[:], AF.Tanh, bias=b_sb[:, oc : oc + 1]
                        )

                    simps = ps_sim.tile([128, ST2], F32, tag="sim")
                    for half in range(2):
                        osl = slice(half * 512, (half + 1) * 512)
                        for oc in range(NOC):
                            nc.tensor.matmul(
                                simps[:, osl],
                                ctx_sb[:, oc * 128 : (oc + 1) * 128],
                                acts[:, oc, osl],
                                start=(oc == 0),
                                stop=(oc == NOC - 1),
                            )

                    ebc = epool.tile([128, ST2], BF16, tag="ebc")
                    nc.scalar.activation(
                        ebc[:], simps[:], AF.Exp,
                        accum_out=den_cols[:, u : u + 1],
                    )

                    for hc in range(NHC):
                        junk = jpool.tile([128, ST2], BF16, tag="junk")
                        nc.vector.affine_mul_reduce(
                            out=junk[:],
                            accum_out=num_cols[:, hc * NP + u : hc * NP + u + 1],
                            in0=x16[:, hc, ssl2],
                            in1=ebc[:],
                            scale=1.0,
                            bias=0.0,
                        )

                # batch epilogue (denominator already per-partition replicated)
                den_col = spool.tile([128, 1], F32, tag="den1")
                nc.vector.reduce_sum(
                    out=den_col[:], in_=den_cols[:], axis=mybir.AxisListType.X
                )
                recip_sb = spool.tile([128, 1], F32, tag="recip")
                nc.vector.reciprocal(recip_sb[:], den_col[:])

                att_red = spool.tile([128, NHC], F32, tag="attred")
                for hc in range(NHC):
                    nc.vector.reduce_sum(
                        out=att_red[:, hc : hc + 1],
                        in_=num_cols[:, hc * NP : (hc + 1) * NP],
                        axis=mybir.AxisListType.X,
                    )
                att_fin = spool.tile([128, NHC], F32, tag="attfin")
                nc.scalar.activation(
                    att_fin[:], att_red[:], AF.Copy, scale=recip_sb[:]
                )

                nc.scalar.dma_start(out_d[b], att_fin[:])

    nc.compile()
    return nc


def _get_nc(mode):
    if mode not in _CACHE:
        _CACHE[mode] = _build(mode)
    return _CACHE[mode]


def _prepare_in_maps(x, W, b, context, mode):
    import ml_dtypes

    x = np.asarray(x, dtype=np.float32)
    W = np.asarray(W, dtype=np.float32)
    b = np.asarray(b, dtype=np.float32)
    context = np.asarray(context, dtype=np.float32)

    # wt[p, hc, o] = W[o, hc*128 + p]
    wt_host = np.ascontiguousarray(
        W.T.reshape(NHC, 128, HID).transpose(1, 0, 2)
    )
    if mode == "fp8":
        wt_host = wt_host.astype(ml_dtypes.float8_e4m3)
    else:
        wt_host = wt_host.astype(ml_dtypes.bfloat16)
    b_host = np.ascontiguousarray(b.reshape(NOC, 128).T)
    # ctxrep[p, oc*128+m] = ctx[oc*128+p]
    ctxrep_host = np.repeat(
        context.reshape(NOC, 128).T[:, :, None], 128, axis=2
    ).reshape(128, NOC * 128).astype(ml_dtypes.bfloat16)
    ctxrep_host = np.ascontiguousarray(ctxrep_host)

    in_maps = []
    for c in range(NCORES):
        xs = x[:, c * BPC : (c + 1) * BPC, :]              # [SEQ, BPC, HID]
        # xt[h_lo, b, hc, s] = x[s, b, hc*128 + h_lo]
        xt = np.ascontiguousarray(
            xs.reshape(SEQ, BPC, NHC, 128).transpose(3, 1, 2, 0)
        )
        m = {
            "x16": xt.astype(ml_dtypes.bfloat16),
            "wt": wt_host,
            "bvec": b_host,
            "ctxrep": ctxrep_host,
        }
        if mode == "fp8":
            m["x8"] = xt.astype(ml_dtypes.float8_e4m3)
        in_maps.append(m)
    return in_maps


def kernel(x, W, b, context, _trace=False):
    from concourse.bass_utils import run_bass_kernel_spmd

    mode = GEMM_MODE
    nc = _get_nc(mode)
    in_maps = _prepare_in_maps(x, W, b, context, mode)

    res = run_bass_kernel_spmd(
        nc, in_maps, core_ids=list(range(NCORES)), trace=_trace
    )
    out = np.concatenate(
        [
            res.results[c]["out"].transpose(0, 2, 1).reshape(BPC, HID)
            for c in range(NCORES)
        ],
        axis=0,
    )
    out = out.astype(np.float32)
    if _trace:
        return out, res
    return out


if __name__ == "__main__":
    rng = np.random.default_rng(0)
    x = rng.standard_normal((SEQ, BATCH, HID), dtype=np.float32)
    W = rng.standard_normal((HID, HID), dtype=np.float32) / np.sqrt(HID)
    b = rng.standard_normal(HID).astype(np.float32) * 0.04
    c = rng.standard_normal(HID).astype(np.float32) * 0.04
    out = kernel(x=x, W=W, b=b, context=c)
    print(out.shape, out.dtype)


# revision 17
# speedup vs baseline: 1.1873x; 1.1873x over previous
"""Additive-attention pooling kernel for TRN2, data-parallel over batch on 8 cores.

Reference computation (per batch b):
    att[s, o]  = tanh(sum_h x[s,b,h] * W[o,h] + bias[o])
    sim[s]     = sum_o att[s, o] * context[o]
    e[s]       = exp(sim[s])            (softmax without max-subtraction;
                                         |sim| <= sum|ctx| <= 22.6 so exp is safe)
    out[b, h]  = sum_s x[s,b,h] * e[s] / sum_s e[s]

V3 dataflow (4 batches/core, pair-units of ST2=1024 seq positions):
  - GEMM: fp8e4m3 DoubleRow matmuls (2 k-chunks per instruction, 2x bf16
    throughput) into a [128, 1024] 2-bank psum per oc; one ACT Tanh per oc
    evacuates the full 1024-wide tile with the per-oc bias (bf16 out).
  - sim: stationary is ctx replicated across 128 columns, so the sim psum
    [128, 1024] holds sim broadcast to every partition.  One ACT Exp
    evacuates it to bf16 e_bc AND emits the denominator partial via
    accum_out - no separate broadcast step.
  - pooling: DVE affine_mul_reduce (fused mul+reduce; tensor_tensor_reduce
    crashes real HW) of x16 chunks against e_bc, accum_out -> num partials.
  - epilogue per batch: reduce partials, reciprocal, scale, DMA out.
"""

import sys

if "/opt/trn_rl_repo" not in sys.path:
    sys.path.insert(0, "/opt/trn_rl_repo")

import numpy as np

SEQ, BATCH, HID = 4096, 32, 512
NCORES = 8
BPC = BATCH // NCORES          # batches per core
ST2 = 1024                     # pair-unit seq width (2 psum banks)
NP = SEQ // ST2                # 4 pair-units per batch
NHC = HID // 128               # 4 h chunks
NOC = HID // 128               # 4 o chunks

# "fp8": x8/W8 e4m3 DoubleRow GEMM (+bf16 x copy for pooling)
# "bf16": bf16 GEMM reading the pooling x copy (no x8)
GEMM_MODE = "fp8"

_CACHE = {}


def _build(mode):
    import concourse.tile as tile
    from concourse import bacc, mybir

    F32 = mybir.dt.float32
    BF16 = mybir.dt.bfloat16
    FP8 = mybir.dt.float8e4
    AF = mybir.ActivationFunctionType
    PM = mybir.MatmulPerfMode

    nc = bacc.Bacc(
        "TRN2",
        target_bir_lowering=False,
        debug=False,
        enable_asserts=True,
        num_devices=NCORES,
    )

    # [h_lo 128, b, hc, s]
    x16_d = nc.dram_tensor("x16", (128, BPC, NHC, SEQ), BF16, kind="ExternalInput").ap()
    if mode == "fp8":
        x8_d = nc.dram_tensor("x8", (128, BPC, NHC, SEQ), FP8, kind="ExternalInput").ap()
        wt_d = nc.dram_tensor("wt", (128, NHC, HID), FP8, kind="ExternalInput").ap()
    else:
        wt_d = nc.dram_tensor("wt", (128, NHC, HID), BF16, kind="ExternalInput").ap()
    b_d = nc.dram_tensor("bvec", (128, NOC), F32, kind="ExternalInput").ap()
    # ctx replicated along free dim: ctxrep[p, oc*128 + m] = ctx[oc*128 + p]
    ctxrep_d = nc.dram_tensor("ctxrep", (128, NOC * 128), BF16, kind="ExternalInput").ap()
    # [b, p, hc] so each batch's out DMA is one contiguous 2KB descriptor;
    # host reassembles out[b, hc*128+p] = out_host[b, p, hc]
    out_d = nc.dram_tensor("out", (BPC, 128, NHC), F32, kind="ExternalOutput").ap()

    with tile.TileContext(nc) as tc:
        with (
            tc.tile_pool(name="consts", bufs=1) as cpool,
            tc.tile_pool(name="xs", bufs=3) as xpool,
            tc.tile_pool(name="acts", bufs=2) as apool,
            tc.tile_pool(name="es", bufs=2) as epool,
            tc.tile_pool(name="junks", bufs=2) as jpool,
            tc.tile_pool(name="small", bufs=2) as spool,
            tc.tile_pool(name="attps", bufs=2, space="PSUM") as ps_att,
            tc.tile_pool(name="simps", bufs=2, space="PSUM") as ps_sim,
        ):
            wdt = FP8 if mode == "fp8" else BF16
            wt_sb = cpool.tile([128, NHC, HID], wdt, tag="wt")
            b_sb = cpool.tile([128, NOC], F32, tag="b")
            ctx_sb = cpool.tile([128, NOC * 128], BF16, tag="ctx")
            nc.sync.dma_start(wt_sb[:], wt_d)
            nc.sync.dma_start(b_sb[:], b_d)
            nc.sync.dma_start(ctx_sb[:], ctxrep_d)

            # PE pstate warm-up: ~12 junk matmuls on a zeroed tile so the
            # first real matmuls run at full clock (PE ramps over ~3us)
            warm = cpool.tile([128, 512], BF16, tag="warm")
            nc.vector.memset(warm[:], 0.0)
            for i in range(6):
                warmps = ps_sim.tile([128, ST2], F32, tag="sim")
                nc.tensor.matmul(
                    warmps[:, 0:512], warm[:, 0:128], warm[:], start=True, stop=True
                )

            for b in range(BPC):
                # chunk the x DMAs per pair-unit so the first att matmuls can
                # start as soon as the first quarter lands (subtile deps);
                # issue from the idle GpSimd queue (cheap issue; the sync
                # queue's DIRECT2D costs ~770ns serial per dma_start)
                x16 = xpool.tile([128, NHC, SEQ], BF16, tag="x16")
                if mode == "fp8":
                    x8 = xpool.tile([128, NHC, SEQ], FP8, tag="x8")
                    # finer leading chunks on batch 0 so compute starts sooner
                    bounds = (
                        [0, 256, 512, 1024, 2048, 3072, 4096]
                        if b == 0
                        else [0, 1024, 2048, 3072, 4096]
                    )
                    for lo, hi in zip(bounds, bounds[1:]):
                        csl = slice(lo, hi)
                        nc.gpsimd.dma_start(x8[:, :, csl], x8_d[:, b, :, csl])
                for u in range(NP):
                    ssl2 = slice(u * ST2, (u + 1) * ST2)
                    nc.gpsimd.dma_start(x16[:, :, ssl2], x16_d[:, b, :, ssl2])

                num_cols = spool.tile([128, NHC * NP], F32, tag="num")
                den_cols = spool.tile([128, NP], F32, tag="den")

                for u in range(NP):
                    ssl2 = slice(u * ST2, (u + 1) * ST2)
                    acts = apool.tile([128, NOC, ST2], BF16, tag="act")
                    for oc in range(NOC):
                        attps = ps_att.tile([128, ST2], F32, tag="att")
                        for half in range(2):
                            osl = slice(half * 512, (half + 1) * 512)
                            ssl = slice(u * ST2 + half * 512, u * ST2 + (half + 1) * 512)
                            if mode == "fp8":
                                for j in range(2):
                                    nc.tensor.matmul(
                                        attps[:, osl],
                                        wt_sb[:, 2 * j : 2 * j + 2,
                                              oc * 128 : (oc + 1) * 128],
                                        x8[:, 2 * j : 2 * j + 2, ssl],
                                        start=(j == 0),
                                        stop=(j == 1),
                                        perf_mode=PM.DoubleRow,
                                    )
                            else:
                                for hc in range(NHC):
                                    nc.tensor.matmul(
                                        attps[:, osl],
                                        wt_sb[:, hc, oc * 128 : (oc + 1) * 128],
                                        x16[:, hc, ssl],
                                        start=(hc == 0),
                                        stop=(hc == NHC - 1),
                                    )
                        nc.scalar.activation(
                            acts[:, oc], attps[:], AF.Tanh, bias=b_sb[:, oc : oc + 1]
                        )

                    simps = ps_sim.tile([128, ST2], F32, tag="sim")
                    for half in range(2):
                        osl = slice(half * 512, (half + 1) * 512)
                        for oc in range(NOC):
                            nc.tensor.matmul(
                                simps[:, osl],
                                ctx_sb[:, oc * 128 : (oc + 1) * 128],
                                acts[:, oc, osl],
                                start=(oc == 0),
                                stop=(oc == NOC - 1),
                            )

                    ebc = epool.tile([128, ST2], BF16, tag="ebc")
                    nc.scalar.activation(
                        ebc[:], simps[:], AF.Exp,
                        accum_out=den_cols[:, u : u + 1],
                    )

                    for hc in range(NHC):
                        junk = jpool.tile([128, ST2], BF16, tag="junk")
                        nc.vector.affine_mul_reduce(
                            out=junk[:],
                            accum_out=num_cols[:, hc * NP + u : hc * NP + u + 1],
                            in0=x16[:, hc, ssl2],
                            in1=ebc[:],
                            scale=1.0,
                            bias=0.0,
                        )

                # batch epilogue (denominator already per-partition replicated)
                den_col = spool.tile([128, 1], F32, tag="den1")
                nc.vector.reduce_sum(
                    out=den_col[:], in_=den_cols[:], axis=mybir.AxisListType.X
                )
                recip_sb = spool.tile([128, 1], F32, tag="recip")
                nc.vector.reciprocal(recip_sb[:], den_col[:])

                att_red = spool.tile([128, NHC], F32, tag="attred")
                for hc in range(NHC):
                    nc.vector.reduce_sum(
                        out=att_red[:, hc : hc + 1],
                        in_=num_cols[:, hc * NP : (hc + 1) * NP],
                        axis=mybir.AxisListType.X,
                    )
                att_fin = spool.tile([128, NHC], F32, tag="attfin")
                nc.scalar.activation(
                    att_fin[:], att_red[:], AF.Copy, scale=recip_sb[:]
                )

                nc.gpsimd.dma_start(out_d[b], att_fin[:])

    nc.compile()
    return nc


def _get_nc(mode):
    if mode not in _CACHE:
        _CACHE[mode] = _build(mode)
    return _CACHE[mode]


def _prepare_in_maps(x, W, b, context, mode):
    import ml_dtypes

    x = np.asarray(x, dtype=np.float32)
    W = np.asarray(W, dtype=np.float32)
    b = np.asarray(b, dtype=np.float32)
    context = np.asarray(context, dtype=np.float32)

    # wt[p, hc, o] = W[o, hc*128 + p]
    wt_host = np.ascontiguousarray(
        W.T.reshape(NHC, 128, HID).transpose(1, 0, 2)
    )
    if mode == "fp8":
        wt_host = wt_host.astype(ml_dtypes.float8_e4m3)
    else:
        wt_host = wt_host.astype(ml_dtypes.bfloat16)
    b_host = np.ascontiguousarray(b.reshape(NOC, 128).T)
    # ctxrep[p, oc*128+m] = ctx[oc*128+p]
    ctxrep_host = np.repeat(
        context.reshape(NOC, 128).T[:, :, None], 128, axis=2
    ).reshape(128, NOC * 128).astype(ml_dtypes.bfloat16)
    ctxrep_host = np.ascontiguousarray(ctxrep_host)

    in_maps = []
    for c in range(NCORES):
        xs = x[:, c * BPC : (c + 1) * BPC, :]              # [SEQ, BPC, HID]
        # xt[h_lo, b, hc, s] = x[s, b, hc*128 + h_lo]
        xt = np.ascontiguousarray(
            xs.reshape(SEQ, BPC, NHC, 128).transpose(3, 1, 2, 0)
        )
        m = {
            "x16": xt.astype(ml_dtypes.bfloat16),
            "wt": wt_host,
            "bvec": b_host,
            "ctxrep": ctxrep_host,
        }
        if mode == "fp8":
            m["x8"] = xt.astype(ml_dtypes.float8_e4m3)
        in_maps.append(m)
    return in_maps


def kernel(x, W, b, context, _trace=False):
    from concourse.bass_utils import run_bass_kernel_spmd

    mode = GEMM_MODE
    nc = _get_nc(mode)
    in_maps = _prepare_in_maps(x, W, b, context, mode)

    res = run_bass_kernel_spmd(
        nc, in_maps, core_ids=list(range(NCORES)), trace=_trace
    )
    out = np.concatenate(
        [
            res.results[c]["out"].transpose(0, 2, 1).reshape(BPC, HID)
            for c in range(NCORES)
        ],
        axis=0,
    )
    out = out.astype(np.float32)
    if _trace:
        return out, res
    return out


if __name__ == "__main__":
    rng = np.random.default_rng(0)
    x = rng.standard_normal((SEQ, BATCH, HID), dtype=np.float32)
    W = rng.standard_normal((HID, HID), dtype=np.float32) / np.sqrt(HID)
    b = rng.standard_normal(HID).astype(np.float32) * 0.04
    c = rng.standard_normal(HID).astype(np.float32) * 0.04
    out = kernel(x=x, W=W, b=b, context=c)
    print(out.shape, out.dtype)


# revision 19
# speedup vs baseline: 1.2019x; 1.0123x over previous
"""Additive-attention pooling kernel for TRN2, data-parallel over batch on 8 cores.

Reference computation (per batch b):
    att[s, o]  = tanh(sum_h x[s,b,h] * W[o,h] + bias[o])
    sim[s]     = sum_o att[s, o] * context[o]
    e[s]       = exp(sim[s])            (softmax without max-subtraction;
                                         |sim| <= sum|ctx| <= 22.6 so exp is safe)
    out[b, h]  = sum_s x[s,b,h] * e[s] / sum_s e[s]

V3 dataflow (4 batches/core, pair-units of ST2=1024 seq positions):
  - GEMM: fp8e4m3 DoubleRow matmuls (2 k-chunks per instruction, 2x bf16
    throughput) into a [128, 1024] 2-bank psum per oc; one ACT Tanh per oc
    evacuates the full 1024-wide tile with the per-oc bias (bf16 out).
  - sim: stationary is ctx replicated across 128 columns, so the sim psum
    [128, 1024] holds sim broadcast to every partition.  One ACT Exp
    evacuates it to bf16 e_bc AND emits the denominator partial via
    accum_out - no separate broadcast step.
  - pooling: DVE affine_mul_reduce (fused mul+reduce; tensor_tensor_reduce
    crashes real HW) of x16 chunks against e_bc, accum_out -> num partials.
  - epilogue per batch: reduce partials, reciprocal, scale, DMA out.
"""

import sys

if "/opt/trn_rl_repo" not in sys.path:
    sys.path.insert(0, "/opt/trn_rl_repo")

import numpy as np

SEQ, BATCH, HID = 4096, 32, 512
NCORES = 8
BPC = BATCH // NCORES          # batches per core
ST2 = 1024                     # pair-unit seq width (2 psum banks)
NP = SEQ // ST2                # 4 pair-units per batch
NHC = HID // 128               # 4 h chunks
NOC = HID // 128               # 4 o chunks

# "fp8": x8/W8 e4m3 DoubleRow GEMM (+bf16 x copy for pooling)
# "bf16": bf16 GEMM reading the pooling x copy (no x8)
GEMM_MODE = "fp8"

_CACHE = {}


def _build(mode):
    import concourse.tile as tile
    from concourse import bacc, mybir

    F32 = mybir.dt.float32
    BF16 = mybir.dt.bfloat16
    FP8 = mybir.dt.float8e4
    AF = mybir.ActivationFunctionType
    PM = mybir.MatmulPerfMode

    nc = bacc.Bacc(
        "TRN2",
        target_bir_lowering=False,
        debug=False,
        enable_asserts=True,
        num_devices=NCORES,
    )

    # [h_lo 128, b, hc, s]
    x16_d = nc.dram_tensor("x16", (128, BPC, NHC, SEQ), BF16, kind="ExternalInput").ap()
    if mode == "fp8":
        x8_d = nc.dram_tensor("x8", (128, BPC, NHC, SEQ), FP8, kind="ExternalInput").ap()
        wt_d = nc.dram_tensor("wt", (128, NHC, HID), FP8, kind="ExternalInput").ap()
    else:
        wt_d = nc.dram_tensor("wt", (128, NHC, HID), BF16, kind="ExternalInput").ap()
    b_d = nc.dram_tensor("bvec", (128, NOC), F32, kind="ExternalInput").ap()
    # ctx replicated along free dim: ctxrep[p, oc*128 + m] = ctx[oc*128 + p]
    ctxrep_d = nc.dram_tensor("ctxrep", (128, NOC * 128), BF16, kind="ExternalInput").ap()
    # [b, p, hc] so each batch's out DMA is one contiguous 2KB descriptor;
    # host reassembles out[b, hc*128+p] = out_host[b, p, hc]
    out_d = nc.dram_tensor("out", (BPC, 128, NHC), F32, kind="ExternalOutput").ap()

    with tile.TileContext(nc) as tc:
        with (
            tc.tile_pool(name="consts", bufs=1) as cpool,
            tc.tile_pool(name="xs", bufs=3) as xpool,
            tc.tile_pool(name="acts", bufs=2) as apool,
            tc.tile_pool(name="es", bufs=2) as epool,
            tc.tile_pool(name="junks", bufs=2) as jpool,
            tc.tile_pool(name="small", bufs=2) as spool,
            tc.tile_pool(name="attps", bufs=2, space="PSUM") as ps_att,
            tc.tile_pool(name="simps", bufs=2, space="PSUM") as ps_sim,
        ):
            wdt = FP8 if mode == "fp8" else BF16
            wt_sb = cpool.tile([128, NHC, HID], wdt, tag="wt")
            b_sb = cpool.tile([128, NOC], F32, tag="b")
            ctx_sb = cpool.tile([128, NOC * 128], BF16, tag="ctx")
            nc.sync.dma_start(wt_sb[:], wt_d)
            nc.sync.dma_start(b_sb[:], b_d)
            nc.sync.dma_start(ctx_sb[:], ctxrep_d)

            # PE pstate warm-up: ~12 junk matmuls on a zeroed tile so the
            # first real matmuls run at full clock (PE ramps over ~3us)
            warm = cpool.tile([128, 512], BF16, tag="warm")
            nc.vector.memset(warm[:], 0.0)
            for i in range(6):
                warmps = ps_sim.tile([128, ST2], F32, tag="sim")
                nc.tensor.matmul(
                    warmps[:, 0:512], warm[:, 0:128], warm[:], start=True, stop=True
                )

            for b in range(BPC):
                # chunk the x DMAs per pair-unit so the first att matmuls can
                # start as soon as the first quarter lands (subtile deps);
                # issue from the idle GpSimd queue (cheap issue; the sync
                # queue's DIRECT2D costs ~770ns serial per dma_start)
                x16 = xpool.tile([128, NHC, SEQ], BF16, tag="x16")
                if mode == "fp8":
                    x8 = xpool.tile([128, NHC, SEQ], FP8, tag="x8")
                    # finer leading chunks on batch 0 so compute starts sooner
                    bounds = (
                        [0, 256, 512, 1024, 2048, 3072, 4096]
                        if b == 0
                        else [0, 1024, 2048, 3072, 4096]
                    )
                    for lo, hi in zip(bounds, bounds[1:]):
                        csl = slice(lo, hi)
                        nc.gpsimd.dma_start(x8[:, :, csl], x8_d[:, b, :, csl])
                for u in range(NP):
                    ssl2 = slice(u * ST2, (u + 1) * ST2)
                    nc.gpsimd.dma_start(x16[:, :, ssl2], x16_d[:, b, :, ssl2])

                num_cols = spool.tile([128, NHC * NP], F32, tag="num")
                den_cols = spool.tile([128, NP], F32, tag="den")

                for u in range(NP):
                    ssl2 = slice(u * ST2, (u + 1) * ST2)
                    acts = apool.tile([128, NOC, ST2], BF16, tag="act")
                    for oc in range(NOC):
                        attps = ps_att.tile([128, ST2], F32, tag="att")
                        for half in range(2):
                            osl = slice(half * 512, (half + 1) * 512)
                            ssl = slice(u * ST2 + half * 512, u * ST2 + (half + 1) * 512)
                            if mode == "fp8":
                                for j in range(2):
                                    nc.tensor.matmul(
                                        attps[:, osl],
                                        wt_sb[:, 2 * j : 2 * j + 2,
                                              oc * 128 : (oc + 1) * 128],
                                        x8[:, 2 * j : 2 * j + 2, ssl],
                                        start=(j == 0),
                                        stop=(j == 1),
                                        perf_mode=PM.DoubleRow,
                                    )
                            else:
                                for hc in range(NHC):
                                    nc.tensor.matmul(
                                        attps[:, osl],
                                        wt_sb[:, hc, oc * 128 : (oc + 1) * 128],
                                        x16[:, hc, ssl],
                                        start=(hc == 0),
                                        stop=(hc == NHC - 1),
                                    )
                        nc.scalar.activation(
                            acts[:, oc], attps[:], AF.Tanh, bias=b_sb[:, oc : oc + 1]
                        )

                    simps = ps_sim.tile([128, ST2], F32, tag="sim")
                    for half in range(2):
                        osl = slice(half * 512, (half + 1) * 512)
                        for oc in range(NOC):
                            nc.tensor.matmul(
                                simps[:, osl],
                                ctx_sb[:, oc * 128 : (oc + 1) * 128],
                                acts[:, oc, osl],
                                start=(oc == 0),
                                stop=(oc == NOC - 1),
                            )

                    ebc = epool.tile([128, ST2], BF16, tag="ebc")
                    nc.scalar.activation(
                        ebc[:], simps[:], AF.Exp,
                        accum_out=den_cols[:, u : u + 1],
                    )

                    for hc in range(NHC):
                        junk = jpool.tile([128, ST2], BF16, tag="junk")
                        nc.vector.affine_mul_reduce(
                            out=junk[:],
                            accum_out=num_cols[:, hc * NP + u : hc * NP + u + 1],
                            in0=x16[:, hc, ssl2],
                            in1=ebc[:],
                            scale=1.0,
                            bias=0.0,
                        )

                # batch epilogue (denominator already per-partition replicated)
                den_col = spool.tile([128, 1], F32, tag="den1")
                nc.vector.reduce_sum(
                    out=den_col[:], in_=den_cols[:], axis=mybir.AxisListType.X
                )
                recip_sb = spool.tile([128, 1], F32, tag="recip")
                nc.vector.reciprocal(recip_sb[:], den_col[:])

                att_red = spool.tile([128, NHC], F32, tag="attred")
                for hc in range(NHC):
                    nc.vector.reduce_sum(
                        out=att_red[:, hc : hc + 1],
                        in_=num_cols[:, hc * NP : (hc + 1) * NP],
                        axis=mybir.AxisListType.X,
                    )
                att_fin = spool.tile([128, NHC], F32, tag="attfin")
                # scale on DVE, not on the bottleneck ACT engine
                nc.vector.tensor_scalar_mul(att_fin[:], att_red[:], recip_sb[:])

                nc.scalar.dma_start(out_d[b], att_fin[:])

    nc.compile()
    return nc


def _get_nc(mode):
    if mode not in _CACHE:
        _CACHE[mode] = _build(mode)
    return _CACHE[mode]


def _prepare_in_maps(x, W, b, context, mode):
    import ml_dtypes

    x = np.asarray(x, dtype=np.float32)
    W = np.asarray(W, dtype=np.float32)
    b = np.asarray(b, dtype=np.float32)
    context = np.asarray(context, dtype=np.float32)

    # wt[p, hc, o] = W[o, hc*128 + p]
    wt_host = np.ascontiguousarray(
        W.T.reshape(NHC, 128, HID).transpose(1, 0, 2)
    )
    if mode == "fp8":
        wt_host = wt_host.astype(ml_dtypes.float8_e4m3)
    else:
        wt_host = wt_host.astype(ml_dtypes.bfloat16)
    b_host = np.ascontiguousarray(b.reshape(NOC, 128).T)
    # ctxrep[p, oc*128+m] = ctx[oc*128+p]
    ctxrep_host = np.repeat(
        context.reshape(NOC, 128).T[:, :, None], 128, axis=2
    ).reshape(128, NOC * 128).astype(ml_dtypes.bfloat16)
    ctxrep_host = np.ascontiguousarray(ctxrep_host)

    in_maps = []
    for c in range(NCORES):
        xs = x[:, c * BPC : (c + 1) * BPC, :]              # [SEQ, BPC, HID]
        # xt[h_lo, b, hc, s] = x[s, b, hc*128 + h_lo]
        xt = np.ascontiguousarray(
            xs.reshape(SEQ, BPC, NHC, 128).transpose(3, 1, 2, 0)
        )
        m = {
            "x16": xt.astype(ml_dtypes.bfloat16),
            "wt": wt_host,
            "bvec": b_host,
            "ctxrep": ctxrep_host,
        }
        if mode == "fp8":
            m["x8"] = xt.astype(ml_dtypes.float8_e4m3)
        in_maps.append(m)
    return in_maps


def kernel(x, W, b, context, _trace=False):
    from concourse.bass_utils import run_bass_kernel_spmd

    mode = GEMM_MODE
    nc = _get_nc(mode)
    in_maps = _prepare_in_maps(x, W, b, context, mode)

    res = run_bass_kernel_spmd(
        nc, in_maps, core_ids=list(range(NCORES)), trace=_trace
    )
    out = np.concatenate(
        [
            res.results[c]["out"].transpose(0, 2, 1).reshape(BPC, HID)
            for c in range(NCORES)
        ],
        axis=0,
    )
    out = out.astype(np.float32)
    if _trace:
        return out, res
    return out


if __name__ == "__main__":
    rng = np.random.default_rng(0)
    x = rng.standard_normal((SEQ, BATCH, HID), dtype=np.float32)
    W = rng.standard_normal((HID, HID), dtype=np.float32) / np.sqrt(HID)
    b = rng.standard_normal(HID).astype(np.float32) * 0.04
    c = rng.standard_normal(HID).astype(np.float32) * 0.04
    out = kernel(x=x, W=W, b=b, context=c)
    print(out.shape, out.dtype)


# revision 25
# speedup vs baseline: 1.2038x; 1.0016x over previous
"""Additive-attention pooling kernel for TRN2, data-parallel over batch on 8 cores.

Reference computation (per batch b):
    att[s, o]  = tanh(sum_h x[s,b,h] * W[o,h] + bias[o])
    sim[s]     = sum_o att[s, o] * context[o]
    e[s]       = exp(sim[s])            (softmax without max-subtraction;
                                         |sim| <= sum|ctx| <= 22.6 so exp is safe)
    out[b, h]  = sum_s x[s,b,h] * e[s] / sum_s e[s]

V3 dataflow (4 batches/core, pair-units of ST2=1024 seq positions):
  - GEMM: fp8e4m3 DoubleRow matmuls (2 k-chunks per instruction, 2x bf16
    throughput) into a [128, 1024] 2-bank psum per oc; one ACT Tanh per oc
    evacuates the full 1024-wide tile with the per-oc bias (bf16 out).
  - sim: stationary is ctx replicated across 128 columns, so the sim psum
    [128, 1024] holds sim broadcast to every partition.  One ACT Exp
    evacuates it to bf16 e_bc AND emits the denominator partial via
    accum_out - no separate broadcast step.
  - pooling: DVE affine_mul_reduce (fused mul+reduce; tensor_tensor_reduce
    crashes real HW) of x16 chunks against e_bc, accum_out -> num partials.
  - epilogue per batch: reduce partials, reciprocal, scale, DMA out.
"""

import sys

if "/opt/trn_rl_repo" not in sys.path:
    sys.path.insert(0, "/opt/trn_rl_repo")

import numpy as np

SEQ, BATCH, HID = 4096, 32, 512
NCORES = 8
BPC = BATCH // NCORES          # batches per core
ST2 = 1024                     # pair-unit seq width (2 psum banks)
NP = SEQ // ST2                # 4 pair-units per batch
NHC = HID // 128               # 4 h chunks
NOC = HID // 128               # 4 o chunks

# "fp8": x8/W8 e4m3 DoubleRow GEMM (+bf16 x copy for pooling)
# "bf16": bf16 GEMM reading the pooling x copy (no x8)
GEMM_MODE = "fp8"

_CACHE = {}


def _build(mode):
    import concourse.tile as tile
    from concourse import bacc, mybir

    F32 = mybir.dt.float32
    BF16 = mybir.dt.bfloat16
    FP8 = mybir.dt.float8e4
    AF = mybir.ActivationFunctionType
    PM = mybir.MatmulPerfMode

    nc = bacc.Bacc(
        "TRN2",
        target_bir_lowering=False,
        debug=False,
        enable_asserts=True,
        num_devices=NCORES,
    )

    # [h_lo 128, b, hc, s]
    x16_d = nc.dram_tensor("x16", (128, BPC, NHC, SEQ), BF16, kind="ExternalInput").ap()
    if mode == "fp8":
        x8_d = nc.dram_tensor("x8", (128, BPC, NHC, SEQ), FP8, kind="ExternalInput").ap()
        wt_d = nc.dram_tensor("wt", (128, NHC, HID), FP8, kind="ExternalInput").ap()
    else:
        wt_d = nc.dram_tensor("wt", (128, NHC, HID), BF16, kind="ExternalInput").ap()
    b_d = nc.dram_tensor("bvec", (128, NOC), F32, kind="ExternalInput").ap()
    # ctx replicated along free dim: ctxrep[p, oc*128 + m] = ctx[oc*128 + p]
    ctxrep_d = nc.dram_tensor("ctxrep", (128, NOC * 128), BF16, kind="ExternalInput").ap()
    # [b, p, hc] so each batch's out DMA is one contiguous 2KB descriptor;
    # host reassembles out[b, hc*128+p] = out_host[b, p, hc]
    out_d = nc.dram_tensor("out", (BPC, 128, NHC), F32, kind="ExternalOutput").ap()

    with tile.TileContext(nc) as tc:
        with (
            tc.tile_pool(name="consts", bufs=1) as cpool,
            tc.tile_pool(name="xs", bufs=3) as xpool,
            tc.tile_pool(name="acts", bufs=2) as apool,
            tc.tile_pool(name="es", bufs=2) as epool,
            tc.tile_pool(name="junks", bufs=2) as jpool,
            tc.tile_pool(name="small", bufs=2) as spool,
            tc.tile_pool(name="attps", bufs=2, space="PSUM") as ps_att,
            tc.tile_pool(name="simps", bufs=2, space="PSUM") as ps_sim,
        ):
            wdt = FP8 if mode == "fp8" else BF16
            wt_sb = cpool.tile([128, NHC, HID], wdt, tag="wt")
            b_sb = cpool.tile([128, NOC], F32, tag="b")
            ctx_sb = cpool.tile([128, NOC * 128], BF16, tag="ctx")
            nc.sync.dma_start(wt_sb[:], wt_d)
            nc.sync.dma_start(b_sb[:], b_d)
            nc.sync.dma_start(ctx_sb[:], ctxrep_d)

            # PE pstate warm-up: junk matmuls on a zeroed tile so the
            # first real matmuls run at full clock (PE ramps over ~3us)
            warm = cpool.tile([128, 512], BF16, tag="warm")
            nc.vector.memset(warm[:], 0.0)
            for i in range(6):
                warmps = ps_sim.tile([128, ST2], F32, tag="sim")
                nc.tensor.matmul(
                    warmps[:, 0:512], warm[:, 0:128], warm[:], start=True, stop=True
                )

            for b in range(BPC):
                # chunk the x DMAs per pair-unit so the first att matmuls can
                # start as soon as the first quarter lands (subtile deps);
                # issue from the idle GpSimd queue (cheap issue; the sync
                # queue's DIRECT2D costs ~770ns serial per dma_start)
                x16 = xpool.tile([128, NHC, SEQ], BF16, tag="x16")
                if mode == "fp8":
                    x8 = xpool.tile([128, NHC, SEQ], FP8, tag="x8")
                    # finer leading chunks on batch 0 so compute starts sooner
                    bounds = (
                        [0, 256, 512, 1024, 2048, 3072, 4096]
                        if b == 0
                        else [0, 1024, 2048, 3072, 4096]
                    )
                    for lo, hi in zip(bounds, bounds[1:]):
                        csl = slice(lo, hi)
                        nc.gpsimd.dma_start(x8[:, :, csl], x8_d[:, b, :, csl])
                for u in range(NP):
                    ssl2 = slice(u * ST2, (u + 1) * ST2)
                    nc.gpsimd.dma_start(x16[:, :, ssl2], x16_d[:, b, :, ssl2])

                num_cols = spool.tile([128, NHC * NP], F32, tag="num")
                den_cols = spool.tile([128, NP], F32, tag="den")

                for u in range(NP):
                    ssl2 = slice(u * ST2, (u + 1) * ST2)
                    acts = apool.tile([128, NOC, ST2], BF16, tag="act")
                    for oc in range(NOC):
                        attps = ps_att.tile([128, ST2], F32, tag="att")
                        for half in range(2):
                            osl = slice(half * 512, (half + 1) * 512)
                            ssl = slice(u * ST2 + half * 512, u * ST2 + (half + 1) * 512)
                            if mode == "fp8":
                                for j in range(2):
                                    nc.tensor.matmul(
                                        attps[:, osl],
                                        wt_sb[:, 2 * j : 2 * j + 2,
                                              oc * 128 : (oc + 1) * 128],
                                        x8[:, 2 * j : 2 * j + 2, ssl],
                                        start=(j == 0),
                                        stop=(j == 1),
                                        perf_mode=PM.DoubleRow,
                                    )
                            else:
                                for hc in range(NHC):
                                    nc.tensor.matmul(
                                        attps[:, osl],
                                        wt_sb[:, hc, oc * 128 : (oc + 1) * 128],
                                        x16[:, hc, ssl],
                                        start=(hc == 0),
                                        stop=(hc == NHC - 1),
                                    )
                        nc.scalar.activation(
                            acts[:, oc], attps[:], AF.Tanh,
                            bias=b_sb[:, oc : oc + 1],
                        )

                    simps = ps_sim.tile([128, ST2], F32, tag="sim")
                    for half in range(2):
                        osl = slice(half * 512, (half + 1) * 512)
                        for oc in range(NOC):
                            nc.tensor.matmul(
                                simps[:, osl],
                                ctx_sb[:, oc * 128 : (oc + 1) * 128],
                                acts[:, oc, osl],
                                start=(oc == 0),
                                stop=(oc == NOC - 1),
                            )

                    ebc = epool.tile([128, ST2], BF16, tag="ebc")
                    nc.scalar.activation(
                        ebc[:], simps[:], AF.Exp,
                        accum_out=den_cols[:, u : u + 1],
                    )
                    for hc in range(NHC):
                        junk = jpool.tile([128, ST2], BF16, tag="junk")
                        nc.vector.affine_mul_reduce(
                            out=junk[:],
                            accum_out=num_cols[:, hc * NP + u : hc * NP + u + 1],
                            in0=x16[:, hc, ssl2],
                            in1=ebc[:],
                            scale=1.0,
                            bias=0.0,
                        )

                # batch epilogue (denominator already per-partition replicated)
                den_col = spool.tile([128, 1], F32, tag="den1")
                nc.vector.reduce_sum(
                    out=den_col[:], in_=den_cols[:], axis=mybir.AxisListType.X
                )
                recip_sb = spool.tile([128, 1], F32, tag="recip")
                nc.vector.reciprocal(recip_sb[:], den_col[:])

                att_red = spool.tile([128, NHC], F32, tag="attred")
                for hc in range(NHC):
                    nc.vector.reduce_sum(
                        out=att_red[:, hc : hc + 1],
                        in_=num_cols[:, hc * NP : (hc + 1) * NP],
                        axis=mybir.AxisListType.X,
                    )
                att_fin = spool.tile([128, NHC], F32, tag="attfin")
                # scale on DVE, not on the bottleneck ACT engine
                nc.vector.tensor_scalar_mul(att_fin[:], att_red[:], recip_sb[:])

                nc.scalar.dma_start(out_d[b], att_fin[:])

    nc.compile()
    return nc


def _get_nc(mode):
    if mode not in _CACHE:
        _CACHE[mode] = _build(mode)
    return _CACHE[mode]


def _prepare_in_maps(x, W, b, context, mode):
    import ml_dtypes

    x = np.asarray(x, dtype=np.float32)
    W = np.asarray(W, dtype=np.float32)
    b = np.asarray(b, dtype=np.float32)
    context = np.asarray(context, dtype=np.float32)

    # wt[p, hc, o] = W[o, hc*128 + p]
    wt_host = np.ascontiguousarray(
        W.T.reshape(NHC, 128, HID).transpose(1, 0, 2)
    )
    if mode == "fp8":
        wt_host = wt_host.astype(ml_dtypes.float8_e4m3)
    else:
        wt_host = wt_host.astype(ml_dtypes.bfloat16)
    b_host = np.ascontiguousarray(b.reshape(NOC, 128).T)
    # ctxrep[p, oc*128+m] = ctx[oc*128+p]
    ctxrep_host = np.repeat(
        context.reshape(NOC, 128).T[:, :, None], 128, axis=2
    ).reshape(128, NOC * 128).astype(ml_dtypes.bfloat16)
    ctxrep_host = np.ascontiguousarray(ctxrep_host)

    in_maps = []
    for c in range(NCORES):
        xs = x[:, c * BPC : (c + 1) * BPC, :]              # [SEQ, BPC, HID]
        # xt[h_lo, b, hc, s] = x[s, b, hc*128 + h_lo]
        xt = np.ascontiguousarray(
            xs.reshape(SEQ, BPC, NHC, 128).transpose(3, 1, 2, 0)
        )
        m = {
            "x16": xt.astype(ml_dtypes.bfloat16),
            "wt": wt_host,
            "bvec": b_host,
            "ctxrep": ctxrep_host,
        }
        if mode == "fp8":
            m["x8"] = xt.astype(ml_dtypes.float8_e4m3)
        in_maps.append(m)
    return in_maps


def kernel(x, W, b, context, _trace=False):
    from concourse.bass_utils import run_bass_kernel_spmd

    mode = GEMM_MODE
    nc = _get_nc(mode)
    in_maps = _prepare_in_maps(x, W, b, context, mode)

    res = run_bass_kernel_spmd(
        nc, in_maps, core_ids=list(range(NCORES)), trace=_trace
    )
    out = np.concatenate(
        [
            res.results[c]["out"].transpose(0, 2, 1).reshape(BPC, HID)
            for c in range(NCORES)
        ],
        axis=0,
    )
    out = out.astype(np.float32)
    if _trace:
        return out, res
    return out


if __name__ == "__main__":
    rng = np.random.default_rng(0)
    x = rng.standard_normal((SEQ, BATCH, HID), dtype=np.float32)
    W = rng.standard_normal((HID, HID), dtype=np.float32) / np.sqrt(HID)
    b = rng.standard_normal(HID).astype(np.float32) * 0.04
    c = rng.standard_normal(HID).astype(np.float32) * 0.04
    out = kernel(x=x, W=W, b=b, context=c)
    print(out.shape, out.dtype)
